# revision 1
# baseline (speedup 1.0000x reference)
"""Trainium2 Bass kernel for nn_MinimalLoss (YOLO-style detection loss).

Strategy (data-parallel over 8 NeuronCores, 4 batches each):
  The only parts of `predictions` [B, HW, 85] that matter are:
    * column 4 (conf logit) of every cell  -> sum of -ln(1-sigmoid(x))
    * the <=200 rows per core addressed by targets -> gathered via
      indirect DMA; xy/wh/cls/conf-correction terms computed on-chip.
  Duplicate-cell targets are deduplicated on-chip (obj_mask semantics of
  the reference scatter-max) with a transpose/is_equal first-occurrence
  matrix. Per-core partial sums (6 scalars) are combined on host.
"""
import os

import numpy as np

import concourse.bass as bass
import concourse.mybir as mybir
import concourse.tile as tile
from concourse.bass import IndirectOffsetOnAxis
from concourse.masks import make_identity

F32 = mybir.dt.float32
I32 = mybir.dt.int32
AF = mybir.ActivationFunctionType
ALU = mybir.AluOpType
AX = mybir.AxisListType

B, HWC, C, T = 32, 25600, 80, 50          # full problem
H = W = 160
NCORES = 8
BL = B // NCORES                          # 4 batches per core
ROWS = BL * HWC                           # 102400 prediction rows per core
NT = BL * T                               # 200 targets per core
HALF = NT // 2                            # 100 targets per half (2 batches)
MAGIC = float(np.float32(2 ** 23))

# conf-channel pass configuration
CONF_VARIANT = os.environ.get("CONF_VARIANT", "strided")  # strided | bulk
NCH = int(os.environ.get("CONF_NCH", "8"))                # strided: chunks of 800/NCH cols
BULK_R = 100                                              # bulk: rows/partition/chunk
CONF_DMA = os.environ.get("CONF_DMA", "sync")           # gpsimd | sync


def _conf_pass_strided(nc, cp, sb, pred_ap, acc):
    """acc[:, k] = per-partition sums of ln(1-sigmoid(conf))."""
    conf = pred_ap[:, 4:5].rearrange("(p j) o -> p (j o)", p=128)  # [128, 800]
    cw = 800 // NCH
    dma_eng = nc.gpsimd if CONF_DMA == "gpsimd" else nc.sync
    for k in range(NCH):
        # dedicated all-live pool: a slot is never reused, so each DMA needs
        # <=1 sync wait (DIRECT2D codegen limit)
        tl = cp.tile([128, cw], F32, tag="conf_in")
        dma_eng.dma_start(out=tl[:], in_=conf[:, k * cw:(k + 1) * cw])
        om = cp.tile([128, cw], F32, tag="conf_om")
        nc.scalar.activation(out=om[:], in_=tl[:], func=AF.Sigmoid)
        nc.vector.tensor_scalar(out=om[:], in0=om[:], scalar1=1.0, scalar2=-1.0,
                                op0=ALU.subtract, op1=ALU.mult)
        ln = cp.tile([128, cw], F32, tag="conf_ln")
        nc.scalar.activation(out=ln[:], in_=om[:], func=AF.Ln, accum_out=acc[:, k:k + 1])


def _conf_pass_bulk(nc, sb, pred_ap, acc):
    """Bulk-load full rows; extract conf with a strided on-chip read."""
    flat = pred_ap.rearrange("r c -> (r c)").rearrange("(p j) -> p j", p=128)  # [128, 800*85]
    nch = 800 // BULK_R
    for k in range(nch):
        tl = sb.tile([128, BULK_R * 85], F32, tag="bulk_in")
        nc.sync.dma_start(out=tl[:], in_=flat[:, k * BULK_R * 85:(k + 1) * BULK_R * 85])
        cv = tl[:].rearrange("p (j c) -> p j c", c=85)[:, :, 4:5].rearrange("p j o -> p (j o)")
        om = sb.tile([128, BULK_R], F32, tag="bulk_om")
        nc.scalar.activation(out=om[:], in_=cv, func=AF.Sigmoid)
        nc.vector.tensor_scalar(out=om[:], in0=om[:], scalar1=1.0, scalar2=-1.0,
                                op0=ALU.subtract, op1=ALU.mult)
        ln = sb.tile([128, BULK_R], F32, tag="bulk_ln")
        nc.scalar.activation(out=ln[:], in_=om[:], func=AF.Ln, accum_out=acc[:, k:k + 1])


def _floor(nc, sb, dst, src, n):
    """dst = floor(src) for 0 <= src < 2^22, exact (round-to-nearest fixup)."""
    r = sb.tile([n, 1], F32, tag="fl_r")
    adj = sb.tile([n, 1], F32, tag="fl_a")
    nc.vector.tensor_scalar_add(r[:], src, MAGIC)
    nc.vector.tensor_scalar_add(r[:], r[:], -MAGIC)
    nc.vector.tensor_tensor(out=adj[:], in0=r[:], in1=src, op=ALU.is_gt)
    nc.vector.tensor_tensor(out=dst, in0=r[:], in1=adj[:], op=ALU.subtract)


def _split_multi_waits(nc):
    """Walrus codegen accepts at most ONE sync wait per instruction; hoist
    extras onto standalone EventSemaphore (wait) ops on the same engine."""
    n = 0
    for func in nc.m.functions:
        for block in func.blocks:
            out = []
            for inst in block.instructions:
                si = inst.sync_info
                if si is not None and si.on_wait and len(si.on_wait) > 1:
                    waits = list(si.on_wait)
                    for w in waits[:-1]:
                        n += 1
                        nop = mybir.InstEventSemaphore(
                            name=f"{inst.name}_sw{n}", engine=inst.engine,
                            ins=[], outs=[])
                        nop.sync_info = mybir.SyncInfo(on_wait=[w], on_update=[])
                        out.append(nop)
                    inst.sync_info = mybir.SyncInfo(on_wait=[waits[-1]],
                                                    on_update=list(si.on_update))
                out.append(inst)
            if n:
                block.instructions[:] = out
    return n


def build_nc(split=True):
    nc = bass.Bass("TRN2", target_bir_lowering=False, debug=False)
    pred_d = nc.dram_tensor("predictions", [ROWS, 85], F32, kind="ExternalInput")
    tgt_d = nc.dram_tensor("targets", [NT, 5], F32, kind="ExternalInput")
    out_d = nc.dram_tensor("out", [8, 1], F32, kind="ExternalOutput")

    pred_ap = pred_d.ap()
    n_conf_cols = NCH if CONF_VARIANT == "strided" else 800 // BULK_R

    with tile.TileContext(nc) as tc:
        with tc.tile_pool(name="persist", bufs=1) as pp, \
             tc.tile_pool(name="conf", bufs=NCH) as cp, \
             tc.tile_pool(name="sb", bufs=2) as sb, \
             tc.tile_pool(name="ps", bufs=1, space="PSUM") as ps:

            acc = pp.tile([128, n_conf_cols], F32)

            # constants (route matmul operands through DVE so each matmul
            # needs at most ONE sync wait — the S3_LW slot limit)
            ident_g = pp.tile([128, 128], F32)
            make_identity(nc, ident_g[:])
            ident = pp.tile([128, 128], F32)
            nc.vector.tensor_copy(out=ident[:], in_=ident_g[:])
            ones = pp.tile([128, 1], F32)
            nc.vector.memset(ones[:], 1.0)
            iotac = pp.tile([128, C], I32)
            nc.gpsimd.iota(iotac[:], pattern=[[1, C]], base=0, channel_multiplier=0)
            iotaf = pp.tile([128, C], F32)
            nc.vector.tensor_copy(out=iotaf[:], in_=iotac[:])
            iotap = pp.tile([128, 1], I32)
            nc.gpsimd.iota(iotap[:], pattern=[[1, 1]], base=0, channel_multiplier=1)
            pf128 = pp.tile([128, 1], F32)
            nc.vector.tensor_copy(out=pf128[:], in_=iotap[:])
            iotar = pp.tile([128, 128], I32)
            nc.gpsimd.iota(iotar[:], pattern=[[1, 128]], base=0, channel_multiplier=0)
            iotarf = pp.tile([128, 128], F32)
            nc.vector.tensor_copy(out=iotarf[:], in_=iotar[:])
            tri = pp.tile([128, 128], F32)  # tri[p, f] = 1.0 iff f < p
            nc.vector.tensor_tensor(out=tri[:], in0=pf128[:].to_broadcast([128, 128]),
                                    in1=iotarf[:], op=ALU.is_gt)

            # ---- conf channel: sum ln(1-sigmoid(x)) over all cells
            if CONF_VARIANT == "strided":
                _conf_pass_strided(nc, cp, sb, pred_ap, acc)
            else:
                _conf_pass_bulk(nc, sb, pred_ap, acc)

            # ---- per-target phase: two halves of 100 targets (2 whole batches each)
            P = HALF
            stats_ps = ps.tile([5, 1], F32, space="PSUM")
            for q in range(2):
                tt = sb.tile([P, 5], F32, tag="tt")
                nc.sync.dma_start(out=tt[:], in_=tgt_d.ap()[q * P:(q + 1) * P, :])

                xW = sb.tile([P, 1], F32, tag="xW")
                yH = sb.tile([P, 1], F32, tag="yH")
                nc.vector.tensor_scalar_mul(xW[:], tt[:, 1:2], float(W))
                nc.vector.tensor_scalar_mul(yH[:], tt[:, 2:3], float(H))
                gx = sb.tile([P, 1], F32, tag="gx")
                gy = sb.tile([P, 1], F32, tag="gy")
                _floor(nc, sb, gx[:], xW[:], P)
                _floor(nc, sb, gy[:], yH[:], P)

                # validity
                vf = sb.tile([P, 1], F32, tag="vf")
                tmp = sb.tile([P, 1], F32, tag="tmp")
                nc.vector.tensor_scalar(out=vf[:], in0=gx[:], scalar1=0.0, scalar2=None, op0=ALU.is_ge)
                nc.vector.tensor_scalar(out=tmp[:], in0=gx[:], scalar1=float(W), scalar2=None, op0=ALU.is_lt)
                nc.vector.tensor_tensor(out=vf[:], in0=vf[:], in1=tmp[:], op=ALU.mult)
                nc.vector.tensor_scalar(out=tmp[:], in0=gy[:], scalar1=0.0, scalar2=None, op0=ALU.is_ge)
                nc.vector.tensor_tensor(out=vf[:], in0=vf[:], in1=tmp[:], op=ALU.mult)
                nc.vector.tensor_scalar(out=tmp[:], in0=gy[:], scalar1=float(H), scalar2=None, op0=ALU.is_lt)
                nc.vector.tensor_tensor(out=vf[:], in0=vf[:], in1=tmp[:], op=ALU.mult)

                # cell + per-core row index
                gxi = sb.tile([P, 1], F32, tag="gxi")
                gyi = sb.tile([P, 1], F32, tag="gyi")
                nc.vector.tensor_scalar(out=gxi[:], in0=gx[:], scalar1=0.0, scalar2=float(W - 1),
                                        op0=ALU.max, op1=ALU.min)
                nc.vector.tensor_scalar(out=gyi[:], in0=gy[:], scalar1=0.0, scalar2=float(H - 1),
                                        op0=ALU.max, op1=ALU.min)
                cell = sb.tile([P, 1], F32, tag="cell")
                nc.vector.tensor_scalar_mul(cell[:], gyi[:], float(W))
                nc.vector.tensor_tensor(out=cell[:], in0=cell[:], in1=gxi[:], op=ALU.add)

                rowf = sb.tile([P, 1], F32, tag="rowf")
                # batch offset: (2q + (t>=50)) * HWC
                nc.vector.tensor_scalar(out=rowf[:], in0=pf128[:P, :], scalar1=float(T), scalar2=None,
                                        op0=ALU.is_ge)
                nc.vector.tensor_scalar(out=rowf[:], in0=rowf[:], scalar1=float(HWC),
                                        scalar2=float(2 * q * HWC), op0=ALU.mult, op1=ALU.add)
                nc.vector.tensor_tensor(out=rowf[:], in0=rowf[:], in1=cell[:], op=ALU.add)
                idx = sb.tile([P, 1], I32, tag="idx")
                nc.vector.tensor_copy(out=idx[:], in_=rowf[:])

                # dedup key: valid -> rowf ; invalid -> unique negative
                negk = sb.tile([P, 1], F32, tag="negk")
                nc.vector.tensor_scalar(out=negk[:], in0=pf128[:P, :], scalar1=-1.0,
                                        scalar2=-(1.0 + 100.0 * q), op0=ALU.mult, op1=ALU.add)
                key = sb.tile([P, 1], F32, tag="key")
                nc.vector.tensor_tensor(out=key[:], in0=rowf[:], in1=negk[:], op=ALU.subtract)
                nc.vector.tensor_tensor(out=key[:], in0=key[:], in1=vf[:], op=ALU.mult)
                nc.vector.tensor_tensor(out=key[:], in0=key[:], in1=negk[:], op=ALU.add)

                # gather prediction rows
                rows = sb.tile([P, 85], F32, tag="rows")
                nc.gpsimd.indirect_dma_start(
                    out=rows[:], out_offset=None, in_=pred_ap[:, :],
                    in_offset=IndirectOffsetOnAxis(ap=idx[:, :1], axis=0))

                # sigmoid/ln terms over the whole row
                sg = sb.tile([P, 85], F32, tag="sg")
                nc.scalar.activation(out=sg[:], in_=rows[:], func=AF.Sigmoid)
                lnp = sb.tile([P, 85], F32, tag="lnp")
                nc.scalar.activation(out=lnp[:], in_=sg[:], func=AF.Ln)
                nc.vector.tensor_scalar_max(lnp[:], lnp[:], -100.0)
                om = sb.tile([P, 85], F32, tag="om")
                nc.vector.tensor_scalar(out=om[:], in0=sg[:], scalar1=1.0, scalar2=-1.0,
                                        op0=ALU.subtract, op1=ALU.mult)
                lnn = sb.tile([P, 85], F32, tag="lnn")
                nc.scalar.activation(out=lnn[:], in_=om[:], func=AF.Ln)
                nc.vector.tensor_scalar_max(lnn[:], lnn[:], -100.0)

                # per_cls = -(1/C) * sum_c [ onehot*lnp + (1-onehot)*lnn ]
                oh = sb.tile([P, C], F32, tag="oh")
                nc.vector.tensor_tensor(out=oh[:], in0=iotaf[:P, :],
                                        in1=tt[:, 0:1].to_broadcast([P, C]), op=ALU.is_equal)
                dlt = sb.tile([P, C], F32, tag="dlt")
                nc.vector.tensor_tensor(out=dlt[:], in0=lnp[:, 5:85], in1=lnn[:, 5:85], op=ALU.subtract)
                nc.vector.tensor_tensor(out=dlt[:], in0=dlt[:], in1=oh[:], op=ALU.mult)
                nc.vector.tensor_tensor(out=dlt[:], in0=dlt[:], in1=lnn[:, 5:85], op=ALU.add)
                pcls = sb.tile([P, 1], F32, tag="pcls")
                nc.vector.reduce_sum(out=pcls[:], in_=dlt[:], axis=AX.X)
                nc.vector.tensor_scalar_mul(pcls[:], pcls[:], -1.0 / C)

                # conf correction term: ct = lnn[4] - lnp[4]  ( = term_pos - term_neg )
                ct = sb.tile([P, 1], F32, tag="ct")
                nc.vector.tensor_tensor(out=ct[:], in0=lnn[:, 4:5], in1=lnp[:, 4:5], op=ALU.subtract)

                # per_xy / per_wh
                txy = sb.tile([P, 2], F32, tag="txy")
                nc.vector.tensor_tensor(out=txy[:, 0:1], in0=xW[:], in1=gx[:], op=ALU.subtract)
                nc.vector.tensor_tensor(out=txy[:, 1:2], in0=yH[:], in1=gy[:], op=ALU.subtract)
                dxy = sb.tile([P, 2], F32, tag="dxy")
                nc.vector.tensor_tensor(out=dxy[:], in0=sg[:, 0:2], in1=txy[:], op=ALU.subtract)
                nc.vector.tensor_tensor(out=dxy[:], in0=dxy[:], in1=dxy[:], op=ALU.mult)
                pxy = sb.tile([P, 1], F32, tag="pxy")
                nc.vector.reduce_sum(out=pxy[:], in_=dxy[:], axis=AX.X)
                nc.vector.tensor_scalar_mul(pxy[:], pxy[:], 0.5)

                pwh_t = sb.tile([P, 2], F32, tag="pwh")
                nc.scalar.activation(out=pwh_t[:], in_=rows[:, 2:4], func=AF.Exp)
                twh = sb.tile([P, 2], F32, tag="twh")
                nc.vector.tensor_scalar_mul(twh[:, 0:1], tt[:, 3:4], float(W))
                nc.vector.tensor_scalar_mul(twh[:, 1:2], tt[:, 4:5], float(H))
                dwh = sb.tile([P, 2], F32, tag="dwh")
                nc.vector.tensor_tensor(out=dwh[:], in0=pwh_t[:], in1=twh[:], op=ALU.subtract)
                nc.vector.tensor_tensor(out=dwh[:], in0=dwh[:], in1=dwh[:], op=ALU.mult)
                pwh = sb.tile([P, 1], F32, tag="pwh1")
                nc.vector.reduce_sum(out=pwh[:], in_=dwh[:], axis=AX.X)
                nc.vector.tensor_scalar_mul(pwh[:], pwh[:], 0.5)

                # dedup: first-occurrence weight w
                keyT_ps = ps.tile([P, P], F32, space="PSUM", tag="keyT_ps")
                nc.tensor.transpose(out=keyT_ps[:], in_=key[:].to_broadcast([P, P]),
                                    identity=ident[:P, :P])
                keyT = sb.tile([P, P], F32, tag="keyT")
                nc.vector.tensor_copy(out=keyT[:], in_=keyT_ps[:])
                eq = sb.tile([P, P], F32, tag="eq")
                nc.vector.tensor_tensor(out=eq[:], in0=key[:].to_broadcast([P, P]),
                                        in1=keyT[:], op=ALU.is_equal)
                nc.vector.tensor_tensor(out=eq[:], in0=eq[:], in1=tri[:P, :P], op=ALU.mult)
                dup = sb.tile([P, 1], F32, tag="dup")
                nc.vector.reduce_max(out=dup[:], in_=eq[:], axis=AX.X)
                wfo = sb.tile([P, 1], F32, tag="wfo")
                nc.vector.tensor_scalar(out=wfo[:], in0=dup[:], scalar1=-1.0, scalar2=1.0,
                                        op0=ALU.mult, op1=ALU.add)
                nc.vector.tensor_tensor(out=wfo[:], in0=wfo[:], in1=vf[:], op=ALU.mult)

                # stats columns: vf*pxy, vf*pwh, vf*pcls, vf, w*ct
                stats = sb.tile([P, 5], F32, tag="stats")
                nc.vector.tensor_tensor(out=stats[:, 0:1], in0=pxy[:], in1=vf[:], op=ALU.mult)
                nc.vector.tensor_tensor(out=stats[:, 1:2], in0=pwh[:], in1=vf[:], op=ALU.mult)
                nc.vector.tensor_tensor(out=stats[:, 2:3], in0=pcls[:], in1=vf[:], op=ALU.mult)
                nc.vector.tensor_copy(out=stats[:, 3:4], in_=vf[:])
                nc.vector.tensor_tensor(out=stats[:, 4:5], in0=ct[:], in1=wfo[:], op=ALU.mult)

                nc.tensor.matmul(out=stats_ps[:], lhsT=stats[:], rhs=ones[:P, :],
                                 start=(q == 0), stop=(q == 1))

            # ---- final reductions
            racc = pp.tile([128, 1], F32)
            nc.vector.reduce_sum(out=racc[:], in_=acc[:], axis=AX.X)
            conf_ps = ps.tile([1, 1], F32, space="PSUM")
            nc.tensor.matmul(out=conf_ps[:], lhsT=ones[:], rhs=racc[:], start=True, stop=True)

            so = pp.tile([5, 1], F32)
            nc.vector.tensor_copy(out=so[:], in_=stats_ps[:])
            co = pp.tile([1, 1], F32)
            nc.vector.tensor_copy(out=co[:], in_=conf_ps[:])
            nc.gpsimd.dma_start(out=out_d.ap()[0:5, :], in_=so[:])
            nc.gpsimd.dma_start(out=out_d.ap()[5:6, :], in_=co[:])
    if split:
        _split_multi_waits(nc)
    return nc


_NC_CACHE = None


def _get_nc():
    global _NC_CACHE
    if _NC_CACHE is None:
        _NC_CACHE = build_nc()
    return _NC_CACHE


def make_in_maps(predictions, targets):
    preds = np.ascontiguousarray(np.asarray(predictions, dtype=np.float32)).reshape(NCORES, ROWS, 85)
    tgts = np.ascontiguousarray(np.asarray(targets, dtype=np.float32)).reshape(NCORES, NT, 5)
    return [{"predictions": preds[c], "targets": tgts[c]} for c in range(NCORES)]


def combine_partials(parts):
    """parts: list of 8 arrays [8,1] -> (total, loss_xy, loss_wh, loss_conf, loss_cls)"""
    s = np.sum([p.reshape(-1) for p in parts], axis=0, dtype=np.float64)
    xy, wh, cls_, nt, corr, lnsum = [np.float32(v) for v in s[:6]]
    denom = np.float32(max(float(nt), 1.0))
    loss_xy = np.float32(xy / denom)
    loss_wh = np.float32(wh / denom)
    loss_cls = np.float32(cls_ / denom)
    loss_conf = np.float32((-lnsum + corr) / np.float32(B * HWC))
    total = np.float32(5.0 * loss_xy + 5.0 * loss_wh + loss_conf + loss_cls)
    return total, loss_xy, loss_wh, loss_conf, loss_cls


def kernel(predictions, targets, H=None, W=None):
    from concourse.bass_utils import run_bass_kernel_spmd

    nc = _get_nc()
    in_maps = make_in_maps(predictions, targets)
    res = run_bass_kernel_spmd(nc, in_maps, core_ids=list(range(NCORES)))
    parts = [res.results[c]["out"] for c in range(NCORES)]
    return combine_partials(parts)



# revision 12
# speedup vs baseline: 3.8095x; 3.8095x over previous
"""Trainium2 Bass kernel for nn_MinimalLoss (YOLO-style detection loss).

Sharding strategy (data-parallel over 8 NeuronCores, 4 batches each):
  * predictions are sharded along B (each core gets its contiguous
    [4*25600, 85] slab, used only for the per-target indirect row gather);
  * the conf channel (column 4) is additionally staged as its own
    contiguous per-core [128, 800] tensor -- a channel-axis shard of
    predictions.  This turns the dominant data access (sum over all cells
    of ln(1-sigmoid(conf))) from a 4-byte-strided DMA (descriptor-rate
    bound, ~78us of SDMA busy) into one 400KB contiguous DMA (~1us).
  * targets are sharded along B and staged slot-packed/field-major as
    [100, 10] so every per-field access on device is a contiguous slice.
  * each core returns raw partial sums ([128, 11]); the final all-reduce
    of the 5 scalar loss terms happens on host in fp64.

Device math (all on-chip):
  * -ln(1-sigmoid(x)) = softplus(x): ONE activation pass with accum_out
    over the conf shard gives per-partition partial sums.
  * conf correction at an object cell: ln(1-s)-ln(s) = -x exactly, so the
    correction is just the gathered conf logit (first-occurrence weighted).
  * bce_cls per target = (sum_c softplus(x_c) - x_cls)/C exactly.
  * pred_xy = sigmoid(rows[:, 0:2]), pred_wh = exp(rows[:, 2:4]) via ACT.
  * duplicate-cell targets deduplicated with transpose/is_equal
    first-occurrence matrix per slot (2 whole batches per slot, so
    duplicates never cross slots).
"""
import numpy as np

import concourse.bass as bass
import concourse.mybir as mybir
import concourse.tile as tile
from concourse.bass import IndirectOffsetOnAxis
from concourse.masks import make_identity

F32 = mybir.dt.float32
I32 = mybir.dt.int32
AF = mybir.ActivationFunctionType
ALU = mybir.AluOpType
AX = mybir.AxisListType

B, HWC, C, T = 32, 25600, 80, 50          # full problem
H = W = 160
NCORES = 8
BL = B // NCORES                          # 4 batches per core
ROWS = BL * HWC                           # 102400 prediction rows per core
NT = BL * T                               # 200 targets per core
P = 100                                   # targets per slot (partition dim)
NS = 2                                    # slots (each = 2 whole batches)
CONF_P, CONF_F = 128, ROWS // 128         # conf shard layout [128, 800]
MAGIC = float(np.float32(2 ** 23))


def _split_multi_waits(nc):
    """Walrus codegen accepts at most ONE sync wait per instruction; hoist
    extras onto standalone EventSemaphore (wait) ops on the same engine."""
    n = 0
    for func in nc.m.functions:
        for block in func.blocks:
            out = []
            for inst in block.instructions:
                si = inst.sync_info
                if si is not None and si.on_wait and len(si.on_wait) > 1:
                    waits = list(si.on_wait)
                    for w in waits[:-1]:
                        n += 1
                        nop = mybir.InstEventSemaphore(
                            name=f"{inst.name}_sw{n}", engine=inst.engine,
                            ins=[], outs=[])
                        nop.sync_info = mybir.SyncInfo(on_wait=[w], on_update=[])
                        out.append(nop)
                    inst.sync_info = mybir.SyncInfo(on_wait=[waits[-1]],
                                                    on_update=list(si.on_update))
                out.append(inst)
            if n:
                block.instructions[:] = out
    return n


def build_nc(split=True):
    nc = bass.Bass("TRN2", target_bir_lowering=False, debug=False)
    pred_d = nc.dram_tensor("predictions", [ROWS, 85], F32, kind="ExternalInput")
    conf_d = nc.dram_tensor("conf", [CONF_P, CONF_F], F32, kind="ExternalInput")
    tgt_d = nc.dram_tensor("targets", [P, NS * 5], F32, kind="ExternalInput")
    out_d = nc.dram_tensor("out", [128, 11], F32, kind="ExternalOutput")

    with tile.TileContext(nc) as tc:
        with tc.tile_pool(name="pp", bufs=1) as pp, \
             tc.tile_pool(name="ps", bufs=1, space="PSUM") as ps:

            # ---- input DMAs, issued first on separate queues
            conf_t = pp.tile([CONF_P, CONF_F], F32)
            nc.sync.dma_start(out=conf_t[:], in_=conf_d.ap())
            tt = pp.tile([P, NS * 5], F32)
            nc.scalar.dma_start(out=tt[:], in_=tgt_d.ap())
            # tt cols (slot-major xy / wh, then cls):
            #   0:4  = {x0,y0,x1,y1}, 4:8 = {w0,h0,w1,h1}, 8:10 = {cls0,cls1}

            # ---- constants (DVE/gpsimd, overlap with the DMAs)
            ident_g = pp.tile([128, 128], F32)
            make_identity(nc, ident_g[:])
            ident = pp.tile([128, 128], F32)
            nc.vector.tensor_copy(out=ident[:], in_=ident_g[:])

            iotac = pp.tile([P, C], I32)
            nc.gpsimd.iota(iotac[:], pattern=[[1, C]], base=0, channel_multiplier=0)
            iotaf = pp.tile([P, C], F32)
            nc.vector.tensor_copy(out=iotaf[:], in_=iotac[:])

            iotap = pp.tile([P, 1], I32)
            nc.gpsimd.iota(iotap[:], pattern=[[1, 1]], base=0, channel_multiplier=1)
            pf = pp.tile([P, 1], F32)
            nc.vector.tensor_copy(out=pf[:], in_=iotap[:])

            iotar = pp.tile([P, P], I32)
            nc.gpsimd.iota(iotar[:], pattern=[[1, P]], base=0, channel_multiplier=0)
            iotarf = pp.tile([P, P], F32)
            nc.vector.tensor_copy(out=iotarf[:], in_=iotar[:])
            tri = pp.tile([P, P], F32)  # tri[p, f] = 1.0 iff f < p
            nc.vector.tensor_tensor(out=tri[:], in0=pf[:].to_broadcast([P, P]),
                                    in1=iotarf[:], op=ALU.is_gt)

            # negk[p, j] = -(1 + p + 100*j): unique negative dedup keys
            negi = pp.tile([P, NS], I32)
            nc.gpsimd.iota(negi[:], pattern=[[P, NS]], base=1, channel_multiplier=1)
            negk = pp.tile([P, NS], F32)
            nc.vector.tensor_copy(out=negk[:], in_=negi[:])
            nc.vector.tensor_scalar_mul(negk[:], negk[:], -1.0)

            # boff[p, j] = HWC * (2j + (p >= 50)): batch row offset
            jci = pp.tile([P, NS], I32)
            nc.gpsimd.iota(jci[:], pattern=[[1, NS]], base=0, channel_multiplier=0)
            boff = pp.tile([P, NS], F32)
            nc.vector.tensor_copy(out=boff[:], in_=jci[:])
            nc.vector.tensor_scalar_mul(boff[:], boff[:], float(2 * HWC))
            par = pp.tile([P, 1], F32)
            nc.vector.tensor_scalar(out=par[:], in0=pf[:], scalar1=float(T),
                                    scalar2=float(HWC), op0=ALU.is_ge, op1=ALU.mult)
            nc.vector.tensor_tensor(out=boff[:], in0=boff[:],
                                    in1=par[:].to_broadcast([P, NS]), op=ALU.add)

            # ---- conf term: sum softplus(conf) = sum ln(1+exp(conf)).
            # Only exp/ln tables are used kernel-wide (one PWP table set, one
            # table load; there is no native softplus table on TRN2).
            csp = pp.tile([CONF_P, 1], F32)
            e_conf = pp.tile([CONF_P, CONF_F], F32)
            nc.scalar.activation(out=e_conf[:], in_=conf_t[:], func=AF.Exp)
            nc.vector.tensor_scalar_add(e_conf[:], e_conf[:], 1.0)
            spdump = pp.tile([CONF_P, CONF_F], F32)
            nc.scalar.activation(out=spdump[:], in_=e_conf[:], func=AF.Ln,
                                 accum_out=csp[:])

            # ---- per-target index chain (slot-major [P, 4] = {x0,y0,x1,y1})
            xyW = pp.tile([P, 4], F32)
            nc.vector.tensor_scalar_mul(xyW[:], tt[:, 0:4], float(W))
            twh = pp.tile([P, 4], F32)
            nc.vector.tensor_scalar_mul(twh[:], tt[:, 4:8], float(W))

            # floor via round-to-nearest magic + fixup
            g_r = pp.tile([P, 4], F32)
            nc.vector.tensor_scalar_add(g_r[:], xyW[:], MAGIC)
            nc.vector.tensor_scalar_add(g_r[:], g_r[:], -MAGIC)
            g_adj = pp.tile([P, 4], F32)
            nc.vector.tensor_tensor(out=g_adj[:], in0=g_r[:], in1=xyW[:], op=ALU.is_gt)
            gxy = pp.tile([P, 4], F32)
            nc.vector.tensor_tensor(out=gxy[:], in0=g_r[:], in1=g_adj[:], op=ALU.subtract)

            # validity: all of (g >= 0) & (g < 160) per slot
            v4 = pp.tile([P, 4], F32)
            t4 = pp.tile([P, 4], F32)
            nc.vector.tensor_scalar(out=v4[:], in0=gxy[:], scalar1=0.0, scalar2=None,
                                    op0=ALU.is_ge)
            nc.vector.tensor_scalar(out=t4[:], in0=gxy[:], scalar1=float(W), scalar2=None,
                                    op0=ALU.is_lt)
            nc.vector.tensor_tensor(out=v4[:], in0=v4[:], in1=t4[:], op=ALU.mult)
            vf = pp.tile([P, NS], F32)
            nc.vector.tensor_tensor(out=vf[:, 0:1], in0=v4[:, 0:1], in1=v4[:, 1:2],
                                    op=ALU.mult)
            nc.vector.tensor_tensor(out=vf[:, 1:2], in0=v4[:, 2:3], in1=v4[:, 3:4],
                                    op=ALU.mult)

            # cell + per-core row index
            gcl = pp.tile([P, 4], F32)
            nc.vector.tensor_scalar(out=gcl[:], in0=gxy[:], scalar1=0.0,
                                    scalar2=float(W - 1), op0=ALU.max, op1=ALU.min)
            cell = pp.tile([P, NS], F32)
            for j in range(NS):
                nc.vector.tensor_scalar(out=cell[:, j:j + 1], in0=gcl[:, 2 * j + 1:2 * j + 2],
                                        scalar1=float(W), scalar2=None, op0=ALU.mult)
                nc.vector.tensor_tensor(out=cell[:, j:j + 1], in0=cell[:, j:j + 1],
                                        in1=gcl[:, 2 * j:2 * j + 1], op=ALU.add)
            rowf = pp.tile([P, NS], F32)
            nc.vector.tensor_tensor(out=rowf[:], in0=cell[:], in1=boff[:], op=ALU.add)
            idx = pp.tile([P, NS], I32)
            nc.vector.tensor_copy(out=idx[:], in_=rowf[:])

            # dedup key: valid -> rowf ; invalid -> unique negative
            key = pp.tile([P, NS], F32)
            nc.vector.tensor_tensor(out=key[:], in0=rowf[:], in1=negk[:], op=ALU.subtract)
            nc.vector.tensor_tensor(out=key[:], in0=key[:], in1=vf[:], op=ALU.mult)
            nc.vector.tensor_tensor(out=key[:], in0=key[:], in1=negk[:], op=ALU.add)

            # ---- gather prediction rows (one indirect DMA per slot)
            rows = [pp.tile([P, 85], F32, name=f"rows{j}") for j in range(NS)]
            for j in range(NS):
                nc.gpsimd.indirect_dma_start(
                    out=rows[j][:], out_offset=None, in_=pred_d.ap()[:, :],
                    in_offset=IndirectOffsetOnAxis(ap=idx[:, j:j + 1], axis=0))

            # ---- per-slot ACT passes (exp/ln only):
            #   sum_c softplus(cls logits) via ln(1+exp(x)) with accum_out;
            #   e4 = exp(xywh logits): wh uses it directly, sigmoid = 1-1/(1+e).
            spc = pp.tile([P, NS], F32)
            sxy = pp.tile([P, 4], F32)   # {sx,sy} per slot
            ewh = pp.tile([P, 4], F32)   # {ew,eh} per slot
            for j in range(NS):
                e80 = pp.tile([P, C], F32, name=f"e80_{j}")
                nc.scalar.activation(out=e80[:], in_=rows[j][:, 5:85], func=AF.Exp)
                nc.vector.tensor_scalar_add(e80[:], e80[:], 1.0)
                spdump2 = pp.tile([P, C], F32, name=f"spdump2_{j}")
                nc.scalar.activation(out=spdump2[:], in_=e80[:], func=AF.Ln,
                                     accum_out=spc[:, j:j + 1])
                e4 = pp.tile([P, 4], F32, name=f"e4_{j}")
                nc.scalar.activation(out=e4[:], in_=rows[j][:, 0:4], func=AF.Exp)
                nc.vector.tensor_copy(out=ewh[:, 2 * j:2 * j + 2], in_=e4[:, 2:4])
                nc.vector.tensor_scalar_add(e4[:, 0:2], e4[:, 0:2], 1.0)
                nc.vector.reciprocal(out=sxy[:, 2 * j:2 * j + 2], in_=e4[:, 0:2])
                nc.vector.tensor_scalar(out=sxy[:, 2 * j:2 * j + 2],
                                        in0=sxy[:, 2 * j:2 * j + 2],
                                        scalar1=-1.0, scalar2=1.0,
                                        op0=ALU.mult, op1=ALU.add)

            # ---- x_cls extraction: onehot dot product per slot
            xcls = pp.tile([P, NS], F32)
            for j in range(NS):
                oh = pp.tile([P, C], F32)
                nc.vector.tensor_tensor(out=oh[:], in0=iotaf[:],
                                        in1=tt[:, 8 + j:9 + j].to_broadcast([P, C]),
                                        op=ALU.is_equal)
                nc.vector.tensor_tensor(out=oh[:], in0=oh[:], in1=rows[j][:, 5:85],
                                        op=ALU.mult)
                nc.vector.reduce_sum(out=xcls[:, j:j + 1], in_=oh[:], axis=AX.X)
            pcls = pp.tile([P, NS], F32)   # = C * per_cls
            nc.vector.tensor_tensor(out=pcls[:], in0=spc[:], in1=xcls[:], op=ALU.subtract)

            # ---- xy / wh squared errors (sum over the 2 coords; /2 on host)
            txy = pp.tile([P, 4], F32)
            nc.vector.tensor_tensor(out=txy[:], in0=xyW[:], in1=gxy[:], op=ALU.subtract)
            dxy = pp.tile([P, 4], F32)
            nc.vector.tensor_tensor(out=dxy[:], in0=sxy[:], in1=txy[:], op=ALU.subtract)
            nc.vector.tensor_tensor(out=dxy[:], in0=dxy[:], in1=dxy[:], op=ALU.mult)
            dwh = pp.tile([P, 4], F32)
            nc.vector.tensor_tensor(out=dwh[:], in0=ewh[:], in1=twh[:], op=ALU.subtract)
            nc.vector.tensor_tensor(out=dwh[:], in0=dwh[:], in1=dwh[:], op=ALU.mult)
            pxy = pp.tile([P, NS], F32)
            pwh = pp.tile([P, NS], F32)
            for j in range(NS):
                nc.vector.tensor_tensor(out=pxy[:, j:j + 1], in0=dxy[:, 2 * j:2 * j + 1],
                                        in1=dxy[:, 2 * j + 1:2 * j + 2], op=ALU.add)
                nc.vector.tensor_tensor(out=pwh[:, j:j + 1], in0=dwh[:, 2 * j:2 * j + 1],
                                        in1=dwh[:, 2 * j + 1:2 * j + 2], op=ALU.add)

            # ---- dedup: first-occurrence weight per slot
            dup = pp.tile([P, NS], F32)
            for j in range(NS):
                keyT_ps = ps.tile([P, P], F32, space="PSUM", tag=f"keyT{j}")
                nc.tensor.transpose(out=keyT_ps[:], in_=key[:, j:j + 1].to_broadcast([P, P]),
                                    identity=ident[:P, :P])
                keyT_sb = pp.tile([P, P], F32)
                nc.vector.tensor_copy(out=keyT_sb[:], in_=keyT_ps[:])
                nc.vector.tensor_tensor(out=keyT_sb[:], in0=key[:, j:j + 1].to_broadcast([P, P]),
                                        in1=keyT_sb[:], op=ALU.is_equal)
                nc.vector.tensor_tensor(out=keyT_sb[:], in0=keyT_sb[:], in1=tri[:], op=ALU.mult)
                nc.vector.reduce_max(out=dup[:, j:j + 1], in_=keyT_sb[:], axis=AX.X)
            wfo = pp.tile([P, NS], F32)
            nc.vector.tensor_scalar(out=wfo[:], in0=dup[:], scalar1=-1.0, scalar2=1.0,
                                    op0=ALU.mult, op1=ALU.add)
            nc.vector.tensor_tensor(out=wfo[:], in0=wfo[:], in1=vf[:], op=ALU.mult)

            # ---- raw per-target stats -> out rows 0:100, cols 0:10
            # cols: [vf*pxy(2), vf*pwh(2), vf*pcls(2), vf(2), w*x4(2)]
            out_t = pp.tile([128, 11], F32)
            nc.vector.memset(out_t[:], 0.0)
            st = out_t[:P, :]
            nc.vector.tensor_tensor(out=st[:, 0:2], in0=pxy[:], in1=vf[:], op=ALU.mult)
            nc.vector.tensor_tensor(out=st[:, 2:4], in0=pwh[:], in1=vf[:], op=ALU.mult)
            nc.vector.tensor_tensor(out=st[:, 4:6], in0=pcls[:], in1=vf[:], op=ALU.mult)
            nc.vector.tensor_copy(out=st[:, 6:8], in_=vf[:])
            for j in range(NS):
                nc.vector.tensor_tensor(out=st[:, 8 + j:9 + j], in0=rows[j][:, 4:5],
                                        in1=wfo[:, j:j + 1], op=ALU.mult)
            nc.vector.tensor_copy(out=out_t[:, 10:11], in_=csp[:])
            nc.sync.dma_start(out=out_d.ap(), in_=out_t[:])
    if split:
        _split_multi_waits(nc)
    return nc


_NC_CACHE = None


def _get_nc():
    global _NC_CACHE
    if _NC_CACHE is None:
        _NC_CACHE = build_nc()
    return _NC_CACHE


def make_in_maps(predictions, targets):
    preds = np.ascontiguousarray(np.asarray(predictions, dtype=np.float32)).reshape(
        NCORES, ROWS, 85)
    # channel-axis shard: conf column staged contiguously per core
    conf = np.ascontiguousarray(preds[:, :, 4]).reshape(NCORES, CONF_P, CONF_F)
    # targets: [NCORES, 4, 50, 5] -> slot-packed [100, (xy slot-major, wh, cls)]
    tg = np.ascontiguousarray(np.asarray(targets, dtype=np.float32)).reshape(
        NCORES, BL, T, 5)
    # z[c, parity, t, j, f] = tg[c, 2j+parity, t, f]
    z = tg.reshape(NCORES, 2, 2, T, 5).transpose(0, 2, 3, 1, 4)  # [c,parity,t,j,f]
    cls_ = z[..., 0]                                   # [c,parity,t,j]
    xy = z[..., 1:3]                                   # [c,parity,t,j,2]
    wh = z[..., 3:5]
    tt = np.concatenate([
        xy.reshape(NCORES, P, 4),                      # {x0,y0,x1,y1}
        wh.reshape(NCORES, P, 4),                      # {w0,h0,w1,h1}
        cls_.reshape(NCORES, P, 2),                    # {cls0,cls1}
    ], axis=2)
    tt = np.ascontiguousarray(tt)
    return [{"predictions": preds[c], "conf": conf[c], "targets": tt[c]}
            for c in range(NCORES)]


def combine_partials(parts):
    """parts: list of 8 arrays [128,11] -> (total, loss_xy, loss_wh, loss_conf, loss_cls)"""
    sxy = swh = scls = nt = corr = spsum = 0.0
    for p in parts:
        a = np.asarray(p, dtype=np.float64)
        st = a[:P, 0:10]
        sxy += st[:, 0:2].sum()
        swh += st[:, 2:4].sum()
        scls += st[:, 4:6].sum()
        nt += st[:, 6:8].sum()
        corr += st[:, 8:10].sum()
        spsum += a[:, 10].sum()
    denom = max(nt, 1.0)
    loss_xy = np.float32(0.5 * sxy / denom)
    loss_wh = np.float32(0.5 * swh / denom)
    loss_cls = np.float32(scls / C / denom)
    loss_conf = np.float32((spsum - corr) / float(B * HWC))
    total = np.float32(5.0 * float(loss_xy) + 5.0 * float(loss_wh)
                       + float(loss_conf) + float(loss_cls))
    return total, loss_xy, loss_wh, loss_conf, loss_cls


def kernel(predictions, targets, H=None, W=None):
    from concourse.bass_utils import run_bass_kernel_spmd

    nc = _get_nc()
    in_maps = make_in_maps(predictions, targets)
    res = run_bass_kernel_spmd(nc, in_maps, core_ids=list(range(NCORES)))
    parts = [res.results[c]["out"] for c in range(NCORES)]
    return combine_partials(parts)
